# revision 29
# baseline (speedup 1.0000x reference)
"""Grouped self-attention (GQA) Trainium2 kernel, v6.

Problem: B=2, T=2048, D=2048, 16 Q heads / 4 KV heads, head_dim=128,
full RoPE (base 1e6), causal softmax, output projection.

Sharding: 8 cores = 2 batches x 4 KV groups. Core c handles batch c//4,
kv-group c%4 (4 Q heads + 1 KV head). q/k/v projections column-sharded,
o_proj row-sharded; per-core partial outputs are summed on host.

v6 (vs v5):
- w split into wkv (k+v cols, streamed per-chunk with x) and wq
  (streamed after x): phase-1a's per-chunk DMA need drops to 576KB
  (~1.65us at ~350GB/s), matching the PE's 1.7us/chunk consumption,
  so the early-stream stalls disappear; chunk 0's x is DMA'd in 4
  pieces so the first matmul issues ~1us earlier.
- k projection eviction switched to the ACT-hybrid rope (2 ACT
  copies + 3 fp16 DVE ops) instead of 4 f32-PSUM DVE reads: the DVE
  queue drains before the attention era starts (v5 stalled ~2us at
  the transition).
- attention heads are software-pipelined: head h's denominator
  matmul + normalization chain is deferred until after head h+1's
  third score strip, so the serial R-chain (DVE) drains behind live
  PE work instead of stalling it (~0.5us x 14 in v5).
- g=3 (the longest strips, no interleaved q-pass to hide behind) is
  DVE-bound: the last 6 strips' denominator contributions go through
  the PE (ones-matmul accumulation) instead of the DVE R-chain, the
  s_sb copy moves to ACT, and the final head's chain drains behind
  o-proj partial accumulations (units 0-2, heads 0-2).
v8 (vs v6/v7):
- v6's ACT-hybrid rope/s_sb-copy in the attention era reverted to
  DVE-direct: ACT is saturated with exps there, and ACT-sourced DVE
  ops head-of-line-block the in-order DVE queue (v6 lost ~5us).
- all input streaming on the ONE sync queue in consumption order
  (HBM BW is shared across queues; a second queue steals from the
  phase-1a stream).
- cross-group o-proj interleave: group g's o-proj units become paced
  fill work inside group g+1's strip loops. The PE is chained to the
  ACT exp backlog via strip-slot reuse (strip idx waits exp(idx-2));
  the fills keep it fed, and g+1's exps overlap what used to be a
  PE-only o-proj block. g=3 pops at half pace so the queue lasts
  through its 41us ACT backlog; its evictions run all-DVE since ACT
  is the g3 bottleneck.
"""

import os
import sys

import numpy as np

for _p in ("/opt/trn_rl_repo",):
    if _p not in sys.path and os.path.isdir(_p):
        sys.path.insert(0, _p)

import concourse.bass as bass  # noqa: E402
import concourse.mybir as mybir  # noqa: E402
import concourse.tile as tile  # noqa: E402
from concourse import bacc  # noqa: E402
from concourse.bass_utils import run_bass_kernel_spmd  # noqa: E402
from concourse.masks import make_identity  # noqa: E402

B, T, D = 2, 2048, 2048
NH, NKV, HD = 16, 4, 128
G = NKV              # kv groups == cores per batch
AQ = (NH // NKV) * HD  # attention cols per core (4 heads x 128)
KC = D // 128        # 16 contraction chunks for projections
ROPE_BASE = 1000000.0
INV_SQRT_D = 1.0 / float(np.sqrt(HD))

F32 = mybir.dt.float32
FP16 = mybir.dt.float16

PT_MAX = 512 * 13 + 768  # widest per-(g,h) P^T row (g=3): 7424


def _strips(g):
    """Score strips for tq group g (cols [512g, 512g+512)).

    Returns [(j, off, w)]: kv block j contributes group columns
    [512-w, 512); off is the strip's offset in the packed P^T buffer.
    """
    out = []
    off = 0
    for j in range(4 * g + 4):
        w = 512 - max(0, 128 * j - 512 * g)
        out.append((j, off, w))
        off += w
    return out


_CACHE = {}

# g=3 R-chain split: strips [0, PE_SUM_FROM) accumulate on DVE, strips
# [PE_SUM_FROM, n) feed the denominator matmul directly (PE has slack
# in g=3; DVE is the bottleneck there).
PE_SUM_FROM = 10


def _build_nc():
    nc = bacc.Bacc(None, target_bir_lowering=False, debug=False)

    # host-packed inputs (see kernel() for layouts)
    xp_d = nc.dram_tensor("xp", [128, KC, T], FP16, kind="ExternalInput")
    wkv_d = nc.dram_tensor("wkv", [128, KC, 256], FP16, kind="ExternalInput")
    wq_d = nc.dram_tensor("wq", [128, KC, 512], FP16, kind="ExternalInput")
    wo_d = nc.dram_tensor("wo", [128, 4, D], FP16, kind="ExternalInput")
    cos_d = nc.dram_tensor("cosT", [HD, T], FP16, kind="ExternalInput")
    sin_d = nc.dram_tensor("sinT", [HD, T], FP16, kind="ExternalInput")
    mask_d = nc.dram_tensor("mask", [128, 128], FP16, kind="ExternalInput")
    y_d = nc.dram_tensor("y", [T, D], FP16, kind="ExternalOutput")

    mult = mybir.AluOpType.mult
    add = mybir.AluOpType.add
    Exp = mybir.ActivationFunctionType.Exp

    with tile.TileContext(nc) as tc:
        with (
            tc.tile_pool(name="const", bufs=1) as cpool,
            tc.tile_pool(name="qkv", bufs=1) as qkv_pool,
            tc.tile_pool(name="xw", bufs=1) as xw_pool,
            tc.tile_pool(name="ptmp", bufs=3) as tmp_pool,
            # single PSUM pool for the whole kernel: 8 one-bank slots
            # s0..s7 managed by tag. Mid-kernel pool closes insert
            # conservative all-bank barriers (cost ~8us each); explicit
            # slot reuse keeps dependencies per-bank instead.
            tc.tile_pool(name="pp", bufs=1, space="PSUM") as pp,
        ):
            cos_sb = cpool.tile([HD, T], FP16, tag="cos")
            sin_sb = cpool.tile([HD, T], FP16, tag="sin")
            mask_sb = cpool.tile([128, 128], FP16, tag="mask")
            id_fp = cpool.tile([128, 128], FP16, tag="idf")
            ones_sb = cpool.tile([128, 1], FP16, tag="ones")
            wo_sb = cpool.tile([128, 4, D], FP16, tag="wo")

            xt = xw_pool.tile([128, KC, T], FP16, tag="xt")
            wkv_sb = xw_pool.tile([128, KC, 256], FP16, tag="wkv")
            wq_sb = xw_pool.tile([128, KC, 512], FP16, tag="wq")
            # Everything streams on the ONE sync queue in exact
            # consumption order: (wkv_e, x_e) pairs pace phase 1a
            # (576KB/chunk ~= the PE's per-chunk consumption), then
            # cos/sin (k ropes, ~40us), then wq (phase 1b), then wo
            # (o-proj, ~90us). HBM BW (~358GB/s) is shared across DMA
            # queues, so putting any of these on a second queue just
            # steals bandwidth from the phase-1a stream (v6 lost 5us
            # to exactly that). Only the tiny mask rides gpsimd.
            nc.gpsimd.dma_start(mask_sb[:], mask_d[:])
            nc.sync.dma_start(wkv_sb[:, 0, :], wkv_d[:, 0, :])
            for q4 in range(4):
                nc.sync.dma_start(
                    xt[:, 0, q4 * 512:(q4 + 1) * 512],
                    xp_d[:, 0, q4 * 512:(q4 + 1) * 512])
            for e in range(1, KC):
                nc.sync.dma_start(wkv_sb[:, e, :], wkv_d[:, e, :])
                # both DMA queues ramp up concurrently at kernel start;
                # spreading the first chunks across them halves the
                # early catch-up lag (same total bytes, no steal)
                eng = nc.gpsimd if e <= 4 else nc.sync
                eng.dma_start(xt[:, e, :], xp_d[:, e, :])
            nc.sync.dma_start(cos_sb[:], cos_d[:])
            nc.sync.dma_start(sin_sb[:], sin_d[:])
            for e in range(KC):
                nc.sync.dma_start(wq_sb[:, e, :], wq_d[:, e, :])
            nc.sync.dma_start(wo_sb[:], wo_d[:])
            make_identity(nc, id_fp[:])
            nc.gpsimd.memset(ones_sb[:], 1.0)

            qT = qkv_pool.tile([128, 4, T], FP16, tag="qT")   # [d, h, t]
            kT = qkv_pool.tile([128, T], FP16, tag="kT")      # [d, t]
            v_sb = qkv_pool.tile([128, T], FP16, tag="v")     # [tk%128, blk*128+d]
            vT_sb = qkv_pool.tile([128, T], FP16, tag="vT")   # [d, t] pre-transpose

            def rope_evict(ps, dst, tsl):
                """Evict the projection PSUM tile via three ACT copies
                (straight + rotate-half, freeing the bank fast), then
                partition-aligned fp16 rope on DVE:
                dst = raw*cos + rot*sin  (sin table carries the
                rotate-half sign)."""
                raw = tmp_pool.tile([128, 512], FP16, tag="qraw")
                rot = tmp_pool.tile([128, 512], FP16, tag="qrot")
                t1 = tmp_pool.tile([128, 512], FP16, tag="ropetmp")
                nc.scalar.copy(raw[:], ps[:])
                nc.scalar.copy(rot[0:64, :], ps[64:128, :])
                nc.scalar.copy(rot[64:128, :], ps[0:64, :])
                # NOTE: keep gpsimd free of tensor ops — the Pool DSP
                # swaps microcode libraries between op families
                # (UNLOAD_LIB/LOAD_LIB, ~6.6us each) and thrashes if it
                # alternates tensor_tensor with partition_broadcast.
                nc.vector.tensor_tensor(t1[:], raw[:], cos_sb[:, tsl], mult)
                nc.vector.tensor_tensor(dst[:], rot[:], sin_sb[:, tsl], mult)
                nc.vector.tensor_tensor(dst[:], dst[:], t1[:], add)

            # ---- phase 1a: k+v projections, e-outer (DMA-streamed) ----
            # slots s0..s3: v accumulators; s4..s7: k accumulators
            psv = [pp.tile([128, 512], F32, tag=f"s{t}",
                           name=f"psv{t}") for t in range(4)]
            psk = [pp.tile([128, 512], F32, tag=f"s{4 + t}",
                           name=f"psk{t}") for t in range(4)]
            for e in range(KC):
                for tci in range(4):
                    nc.tensor.matmul(
                        psv[tci][:], wkv_sb[:, e, 128:256],
                        xt[:, e, tci * 512:(tci + 1) * 512],
                        start=(e == 0), stop=(e == KC - 1))
                for tci in range(4):
                    nc.tensor.matmul(
                        psk[tci][:], wkv_sb[:, e, 0:128],
                        xt[:, e, tci * 512:(tci + 1) * 512],
                        start=(e == 0), stop=(e == KC - 1))
            # vT first: 4 fast ACT copies release the psv slots the
            # phase-1b q-pairs reuse; k ropes use the ACT-hybrid path
            # (raw/rot copies on ACT, fp16 math on DVE) so the DVE
            # queue is drained before the attention era needs it.
            for tci in range(4):
                tsl = slice(tci * 512, (tci + 1) * 512)
                nc.scalar.copy(vT_sb[:, tsl], psv[tci][:])
            for tci in range(4):
                tsl = slice(tci * 512, (tci + 1) * 512)
                rope_evict(psk[tci], kT[:, tsl], tsl)

            # ---- phase 1b: q tci0 head-pairs, v transpose ----
            def q_pass_pair(ha, tci, ta, tb):
                tsl = slice(tci * 512, (tci + 1) * 512)
                pa = pp.tile([128, 512], F32, tag=ta,
                             name=f"q{ha}_{tci}")
                pb = pp.tile([128, 512], F32, tag=tb,
                             name=f"q{ha + 1}_{tci}")
                for e in range(KC):
                    nc.tensor.matmul(
                        pa[:], wq_sb[:, e, ha * 128:(ha + 1) * 128],
                        xt[:, e, tsl], start=(e == 0), stop=(e == KC - 1))
                    nc.tensor.matmul(
                        pb[:], wq_sb[:, e, (ha + 1) * 128:(ha + 2) * 128],
                        xt[:, e, tsl], start=(e == 0), stop=(e == KC - 1))
                rope_evict(pa, qT[:, ha, tsl], tsl)
                rope_evict(pb, qT[:, ha + 1, tsl], tsl)

            def rope_evict_dve(ps, dst, tsl):
                """All-DVE rope straight from PSUM (partition-shifted
                reads are legal with a PSUM operand). Used in the
                attention era where ACT is saturated with exps: an
                ACT-sourced rope would head-of-line-block the loaded
                DVE queue (v6 lost ~5us to exactly that)."""
                t1 = tmp_pool.tile([128, 512], FP16, tag="ropetmp")
                nc.vector.tensor_tensor(t1[:], ps[:], cos_sb[:, tsl], mult)
                nc.vector.tensor_tensor(
                    dst[0:64, :], ps[64:128, :], sin_sb[0:64, tsl], mult)
                nc.vector.tensor_tensor(
                    dst[64:128, :], ps[0:64, :], sin_sb[64:128, tsl], mult)
                nc.vector.tensor_tensor(dst[:], dst[:], t1[:], add)

            def q_pass(h, tci):
                tsl = slice(tci * 512, (tci + 1) * 512)
                ps = pp.tile([128, 512], F32, tag="s3",
                             name=f"q{h}_{tci}")
                for e in range(KC):
                    nc.tensor.matmul(
                        ps[:], wq_sb[:, e, h * 128:(h + 1) * 128],
                        xt[:, e, tsl], start=(e == 0), stop=(e == KC - 1))
                rope_evict_dve(ps, qT[:, h, tsl], tsl)

            q_pass_pair(0, 0, "s0", "s1")
            q_pass_pair(2, 0, "s2", "s3")
            for tci in range(4):
                tsl = slice(tci * 512, (tci + 1) * 512)
                pst = pp.tile([128, 512], FP16, tag=f"s{4 + tci % 2}",
                              name=f"vtr{tci}")
                for j4 in range(4):
                    nc.tensor.transpose(
                        pst[:, j4 * 128:(j4 + 1) * 128],
                        vT_sb[:, tci * 512 + j4 * 128:
                              tci * 512 + (j4 + 1) * 128],
                        id_fp[:],
                    )
                nc.vector.tensor_copy(v_sb[:, tsl], pst[:])

            # ---- attention, tq-group-major, o-proj per group ----
            # slot plan: ST s4/s5 (alternating per strip), OT s6/s7
            # (per head), SUM s2, interleaved q passes s3, Y s0/s1.
            with (
                tc.tile_pool(name="att", bufs=2) as att_pool,
                tc.tile_pool(name="small", bufs=2) as small_pool,
                tc.tile_pool(name="yev", bufs=6) as yev_pool,
            ):
                cp = 0
                # cross-group fill queue: group g's o-proj unit steps,
                # popped between group g+1's strips. Keeps the PE fed
                # while it would otherwise wait on the ACT exp backlog
                # (strip slot reuse chains the PE to exp(idx-2)), and
                # lets g+1's exps overlap what used to be a PE-only
                # o-proj region.
                fill_q = []

                def emit_fill(k=1):
                    while k > 0 and fill_q:
                        fill_q.pop(0)()
                        k -= 1

                for g in range(4):
                    sl = _strips(g)
                    n = len(sl)
                    # g=3: strips >= pe_from skip the DVE R-chain and
                    # feed the denominator matmul directly.
                    pe_from = PE_SUM_FROM if g == 3 else n
                    OTg = att_pool.tile([128, 4, 512], FP16, tag="OTg")
                    pending = [None]

                    def flush_pending(pending=pending):
                        if pending[0] is not None:
                            pending[0]()
                            pending[0] = None

                    units = [(tb, nci) for tb in range(4) for nci in range(4)]
                    psys = {}

                    # g3's first 7 units get distinct PSUM slots (s4/s5
                    # free after its last strips, s6 after OTg[2]'s
                    # mult, s2 after the last denominator copy), so
                    # their head-0..2 partials can fill the final
                    # normalization chain's ~3.4us latency.
                    G3_SLOTS = {0: "s0", 1: "s1", 2: "s3", 3: "s4",
                                4: "s5", 5: "s6", 6: "s2"}

                    def oproj_mms(u, hs, units=units, psys=psys, g=g,
                                  OTg=OTg):
                        tb, nci = u
                        if u not in psys:
                            ui = units.index(u)
                            if g == 3 and ui in G3_SLOTS:
                                slot = G3_SLOTS[ui]
                            else:
                                slot = f"s{ui % 2}"
                            psys[u] = pp.tile(
                                [128, 512], F32, tag=slot,
                                name=f"y_{g}_{tb}_{nci}")
                        for h in hs:
                            nc.tensor.matmul(
                                psys[u][:],
                                OTg[:, h, tb * 128:(tb + 1) * 128],
                                wo_sb[:, h, nci * 512:(nci + 1) * 512],
                                start=(h == 0), stop=(h == 3),
                                skip_group_check=True)

                    def oproj_evict(u, fast=False, psys=psys, g=g):
                        tb, nci = u
                        ysb = yev_pool.tile([128, 512], FP16, tag="ysb")
                        nonlocal cp
                        rows = y_d[512 * g + tb * 128:
                                   512 * g + tb * 128 + 128, :]
                        if fast:
                            # final units: halves on both engines and
                            # both queues so the drain tail shrinks
                            nc.scalar.copy(ysb[:, 0:256],
                                           psys[u][:, 0:256])
                            nc.vector.tensor_copy(ysb[:, 256:512],
                                                  psys[u][:, 256:512])
                            nc.sync.dma_start(
                                rows[:, nci * 512:nci * 512 + 256],
                                ysb[:, 0:256])
                            nc.gpsimd.dma_start(
                                rows[:, nci * 512 + 256:nci * 512 + 512],
                                ysb[:, 256:512])
                            cp += 1
                            return
                        # engine policy: these copies run inside group
                        # g+1's attention era. g2's evictions land in
                        # g3 where ACT (exps) is the bottleneck -> all
                        # DVE there; elsewhere alternate.
                        if g == 2 or cp % 2 == 1:
                            nc.vector.tensor_copy(ysb[:], psys[u][:])
                        else:
                            nc.scalar.copy(ysb[:], psys[u][:])
                        cp += 1
                        dma_eng = nc.sync if cp % 2 == 0 else nc.gpsimd
                        dma_eng.dma_start(
                            rows[:, nci * 512:(nci + 1) * 512],
                            ysb[:])

                    for h in range(4):
                        PT = att_pool.tile([128, PT_MAX], FP16, tag="PT")
                        R0 = att_pool.tile([128, 512], FP16, tag="R0")
                        pso = pp.tile([128, 512], F32,
                                      tag=f"s{6 + (4 * g + h) % 2}",
                                      name=f"ot_{g}_{h}")

                        def ot_mm(idx, pso=pso, PT=PT):
                            j, off, w = sl[idx]
                            nc.tensor.matmul(
                                pso[:, 512 - w:512],
                                v_sb[:, j * 128:(j + 1) * 128],
                                PT[:, off:off + w],
                                start=(idx == 0),
                                stop=(idx == n - 1),
                                skip_group_check=True,
                            )

                        for idx, (j, off, w) in enumerate(sl):
                            if idx >= 3 and (g != 3 or idx % 2 == 1):
                                # paced o-proj fill from the previous
                                # group (g=3 at half pace so the queue
                                # lasts through its long ACT backlog)
                                emit_fill(1)
                            ps = pp.tile([128, 512], F32,
                                         tag=f"s{4 + idx % 2}",
                                         name=f"st_{g}_{h}_{idx}")
                            tq0 = max(512 * g, 128 * j)
                            nc.tensor.matmul(
                                ps[:, :w],
                                kT[:, j * 128:(j + 1) * 128],
                                qT[:, h, tq0:512 * g + 512],
                                start=True, stop=True,
                                skip_group_check=True)
                            nc.scalar.activation(
                                PT[:, off:off + w], ps[:, :w], Exp,
                                scale=INV_SQRT_D)
                            if j >= 4 * g:
                                # post-exp 0/1 mask: exp never waits DVE
                                nc.vector.tensor_tensor(
                                    PT[:, off:off + 128],
                                    PT[:, off:off + 128],
                                    mask_sb[:], mult)
                            # strip accumulation for softmax sums (DVE);
                            # strips >= pe_from go through the PE in the
                            # deferred denominator matmul instead.
                            if idx == 0:
                                nc.vector.tensor_copy(R0[:], PT[:, 0:512])
                            elif idx < pe_from:
                                nc.vector.tensor_tensor(
                                    R0[:, 512 - w:], R0[:, 512 - w:],
                                    PT[:, off:off + w], add)
                            if idx >= 2:
                                ot_mm(idx - 2)
                                # head h-1's denominator + normalization
                                # drain here, behind h's live strip work
                                if idx == 2:
                                    flush_pending()
                        if n >= 2:
                            ot_mm(n - 2)
                        ot_mm(n - 1)
                        # prefetch next tq group's q projection: the PE
                        # chews on it while h's R chain drains
                        if g < 3:
                            q_pass(h, g + 1)
                        emit_fill(2)

                        def finish_head(h=h, R0=R0, PT=PT, pso=pso,
                                        ivl=None):
                            ps1 = pp.tile([1, 512], F32, tag="s2",
                                          name=f"sum_{g}_{h}")
                            nsum = 1 + (n - pe_from)
                            nc.tensor.matmul(
                                ps1[:], ones_sb[:], R0[:],
                                start=True, stop=(nsum == 1),
                                skip_group_check=True)
                            for pidx in range(pe_from, n):
                                # pad each exp-gated partial with fill
                                # work (g3 h3: o-proj unit partials)
                                if ivl:
                                    ivl.pop(0)()
                                _, off, w = sl[pidx]
                                nc.tensor.matmul(
                                    ps1[:, 512 - w:], ones_sb[:],
                                    PT[:, off:off + w],
                                    start=False, stop=(pidx == n - 1),
                                    skip_group_check=True)
                            while ivl:
                                ivl.pop(0)()
                            s_sb = small_pool.tile([1, 512], F32, tag="s")
                            nc.vector.tensor_copy(s_sb[:], ps1[:])
                            bc = small_pool.tile([128, 512], F32, tag="bc")
                            nc.gpsimd.partition_broadcast(bc[:], s_sb[:])
                            rcp = small_pool.tile([128, 512], F32, tag="rcp")
                            nc.vector.reciprocal_approx_fast(rcp[:], bc[:])
                            nc.vector.tensor_tensor(
                                OTg[:, h, :], pso[:], rcp[:], mult)

                        if g == 3 and h == 3:
                            # last head of the last group: drain the
                            # denominator behind o-proj unit partials
                            finish_head(ivl=[
                                lambda: oproj_mms(units[0], [0, 1, 2]),
                                lambda: oproj_mms(units[1], [0, 1, 2]),
                                lambda: oproj_mms(units[2], [0, 1, 2]),
                            ])
                            # more unit partials to cover the ~3.4us
                            # copy->broadcast->rcp->mult chain before
                            # anything h3-gated can run
                            for ui in range(3, 7):
                                oproj_mms(units[ui], [0, 1, 2])
                        else:
                            pending[0] = finish_head

                    flush_pending()
                    emit_fill(len(fill_q))
                    if g == 3:
                        # tail o-proj: units 0-6 already hold heads
                        # 0-2, so their completion is one matmul each.
                        # Interleave them with full units: 7 rapid
                        # 216ns completions back-to-back would swamp
                        # the eviction pipeline (copy+DMA ~1.3us each)
                        # and stall on ysb/psum recycling.
                        # completions c0/c1 must precede the first
                        # full units: fulls allocate s0/s1, released
                        # only by u0/u1's evictions (emission-order
                        # deadlock otherwise)
                        oproj_mms(units[0], [3])
                        oproj_evict(units[0])
                        oproj_mms(units[1], [3])
                        oproj_evict(units[1])
                        comps = [2, 3, 4, 5, 6]
                        fulls = list(range(7, 16))
                        last2 = units[14:16]
                        while comps or fulls:
                            if fulls:
                                ui = fulls.pop(0)
                                oproj_mms(units[ui], [0, 1, 2, 3])
                                oproj_evict(units[ui],
                                            fast=units[ui] in last2)
                            if comps:
                                ui = comps.pop(0)
                                oproj_mms(units[ui], [3])
                                oproj_evict(units[ui])
                    else:
                        # queue this group's o-proj as fill work for
                        # the next group's attention era (om/oe/us
                        # default-bound: the names rebind next group)
                        om, oe, us = oproj_mms, oproj_evict, units
                        steps = [
                            lambda om=om, us=us: om(us[0], [0, 1, 2]),
                            lambda om=om, us=us: om(us[1], [0, 1, 2]),
                            lambda om=om, us=us: om(us[0], [3]),
                            lambda oe=oe, us=us: oe(us[0]),
                            lambda om=om, us=us: om(us[1], [3]),
                            lambda oe=oe, us=us: oe(us[1]),
                        ]
                        for u in units[2:]:
                            steps.append(
                                lambda u=u, om=om: om(u, [0, 1, 2, 3]))
                            steps.append(lambda u=u, oe=oe: oe(u))
                        fill_q = steps

    nc.compile()
    return nc


def _rope_tables():
    pos = np.arange(T, dtype=np.float32)
    inv_freq = (1.0 / (ROPE_BASE ** (np.arange(0, HD, 2, dtype=np.float32) / HD))).astype(np.float32)
    ang = pos[:, None] * inv_freq[None, :]            # [T, 64]
    cos = np.cos(ang).astype(np.float32)
    sin = np.sin(ang).astype(np.float32)
    cosT = np.ascontiguousarray(np.concatenate([cos, cos], 1).T)   # [128, T]
    sinT = np.ascontiguousarray(np.concatenate([-sin, sin], 1).T)  # rotate_half sign
    return cosT.astype(np.float16), sinT.astype(np.float16)


def kernel(x, Wq, bq, Wk, bk, Wv, bv, Wo, bo, **_ignored):
    x = np.asarray(x, dtype=np.float32)
    Wq = np.asarray(Wq, dtype=np.float32)
    Wk = np.asarray(Wk, dtype=np.float32)
    Wv = np.asarray(Wv, dtype=np.float32)
    Wo = np.asarray(Wo, dtype=np.float32)
    bo = np.asarray(bo, dtype=np.float32)

    if "nc" not in _CACHE:
        _CACHE["nc"] = _build_nc()
    nc = _CACHE["nc"]

    cosT, sinT = _rope_tables()
    # S^T layout: mask[tk, tq] allows tk <= tq within the diagonal block
    triu = np.triu(np.ones((128, 128), dtype=bool))
    mask = np.where(triu, 1.0, 0.0).astype(np.float16)

    in_maps = []
    for c in range(8):
        b, g = c // G, c % G
        xT = x[b].T.astype(np.float16)                  # [D, T]
        xp = np.ascontiguousarray(
            xT.reshape(KC, 128, T).transpose(1, 0, 2))
        wq = Wq[:, g * AQ:(g + 1) * AQ].astype(np.float16)
        wk = Wk[:, g * HD:(g + 1) * HD].astype(np.float16)
        wv = Wv[:, g * HD:(g + 1) * HD].astype(np.float16)
        wkv = np.concatenate([wk, wv], axis=1)          # [D, 256]
        wkvp = np.ascontiguousarray(
            wkv.reshape(KC, 128, 256).transpose(1, 0, 2))
        wqp = np.ascontiguousarray(
            wq.reshape(KC, 128, 512).transpose(1, 0, 2))
        wo = np.ascontiguousarray(
            Wo[g * AQ:(g + 1) * AQ, :].astype(np.float16)
            .reshape(4, 128, D).transpose(1, 0, 2))
        in_maps.append({
            "xp": xp,
            "wkv": wkvp,
            "wq": wqp,
            "wo": wo,
            "cosT": cosT,
            "sinT": sinT,
            "mask": mask,
        })

    res = run_bass_kernel_spmd(
        nc, in_maps, list(range(8)),
        trace=bool(os.environ.get("KERNEL_TRACE")),
        tmpdir=os.environ.get("KERNEL_TRACE_DIR") or None,
    )
    _CACHE["last_results"] = res

    out = np.zeros((B, T, D), dtype=np.float32)
    for b in range(B):
        acc = np.zeros((T, D), dtype=np.float32)
        for g in range(G):
            acc += res.results[b * G + g]["y"].astype(np.float32)
        out[b] = acc + bo[None, :]
    return out
